# revision 16
# baseline (speedup 1.0000x reference)
"""Trainium2 Bass kernel for nn_Cross_Domain_Class_Alignment.

Reference computation (per sample b):
    mask0[b] = argmin_k || feature_s2t[b,:,r,c] - centroid_target[k] ||^2
    mask1[b] = argmin_k || feature_target[b,:,r,c] - centroid_s2t[k] ||^2
    both nearest-upsampled from (65,129) to (512,1024).

Sharding: data-parallel over batch B=8 across 8 NeuronCores (1 sample/core).
Centroids are replicated.

v2 dataflow (per core, per mask), built to keep HBM at roofline:
  - features are cast to fp16 on the HOST (halves input DMA traffic; the
    argmin decision noise this adds is ~1e-4 mismatch, measured well under
    the 2e-2 gate), streamed in 4 quads of 2048 px x 2 channel chunks.
  - dist matmuls, centroid-stationary: psum quad [128, 512] holds four
    512-px banks at partition offsets {0,32,64,96} via tile_position
    col-tiling.  Bank j takes the INTERLEAVED pixel set {128c+32j+i} so
    that after a DVE 32x32 stream-transpose the quad lands in flat
    pixel-block order (pixel = 128*block + partition).
  - scalar-engine copy fuses m = 2*dots - csq (per-partition bias) while
    moving PSUM->SBUF; DVE stream-transpose flips pixels onto partitions
    (no PE transposes in the stream).
  - batched DVE argmin: reduce_max / is_ge / *(k-20) / reduce_min gives
    y = idx - 20 with first-index tie-break.
  - PE transpose of y blocks + DRAM bounce reshapes flat pixel order to
    [65, 129]; column nearest-upsample (129->1024) via segmented DVE
    broadcast adds +20 and emits idx as bf16.
  - row nearest-upsample via one-hot gather matmul (bf16) -> int8 SBUF ->
    int8 DRAM store (host widens to int32).
  - all finish work is pipelined into the feature stream per availability;
    only the last output chunk (rows 384:512 need input row 64 = the 193-px
    remainder) trails the final input bytes.
"""

import numpy as np

B, C, h, w = 8, 256, 65, 129
K = 19
H, W = 512, 1024
HW = h * w              # 8385
QUAD_PX = 2048
NFULL = HW // QUAD_PX   # 4 full quads
REM = HW - NFULL * QUAD_PX   # 193 remainder pixels
REMPAD = 256
NT = (HW + 127) // 128  # 66 pixel blocks of 128


def _col_segments():
    """Segments of the nearest-neighbor column map ci[c'] = c'*129 // 1024."""
    ci = (np.arange(W) * w) // W
    reps = np.bincount(ci, minlength=w)
    segs = []
    i, dst = 0, 0
    while i < w:
        j = i
        while j < w and reps[j] == reps[i]:
            j += 1
        segs.append((i, j - i, int(reps[i]), dst))
        dst += (j - i) * int(reps[i])
        i = j
    assert dst == W
    return segs


def _row_onehot():
    """G[s, r'] = 1.0 iff floor(r'*65/512) == s; shape [65, 512] bf16."""
    import ml_dtypes

    ri = (np.arange(H) * h) // H
    return (ri[None, :] == np.arange(h)[:, None]).astype(ml_dtypes.bfloat16)


def build_module(num_devices=8):
    import concourse.bass as bass
    import concourse.tile as tile
    from concourse import bacc, mybir
    from concourse.ap import AP

    f32 = mybir.dt.float32
    f16 = mybir.dt.float16
    bf16 = mybir.dt.bfloat16
    i8 = mybir.dt.int8

    nc = bacc.Bacc(
        "TRN2",
        target_bir_lowering=False,
        debug=False,
        enable_asserts=False,
        num_devices=num_devices,
    )

    f_s2t = nc.dram_tensor("feature_s2t", [C, HW], f16, kind="ExternalInput")
    f_tgt = nc.dram_tensor("feature_target", [C, HW], f16, kind="ExternalInput")
    centT0_d = nc.dram_tensor("centT0", [C, 32], f16, kind="ExternalInput")
    centT1_d = nc.dram_tensor("centT1", [C, 32], f16, kind="ExternalInput")
    csqn0_d = nc.dram_tensor("csqn0", [128, 1], f32, kind="ExternalInput")
    csqn1_d = nc.dram_tensor("csqn1", [128, 1], f32, kind="ExternalInput")
    out0 = nc.dram_tensor("out0", [H, W], i8, kind="ExternalOutput")
    out1 = nc.dram_tensor("out1", [H, W], i8, kind="ExternalOutput")

    ident_dram = nc.inline_tensor(np.eye(128, dtype=np.float32), name="ident_const")
    g_dram = nc.inline_tensor(_row_onehot(), name="rowgather_const")
    wk_np = np.tile((np.arange(K) - 20.0).astype(np.float32), (128, 1))
    wk_dram = nc.inline_tensor(wk_np, name="wk_const")

    col_segs = _col_segments()
    X = mybir.AxisListType.X
    ALU = mybir.AluOpType
    AF = mybir.ActivationFunctionType

    with tile.TileContext(nc) as tc:
        from contextlib import ExitStack

        with ExitStack() as ctx:
            const_p = ctx.enter_context(tc.tile_pool(name="const", bufs=1))
            feat_p = ctx.enter_context(tc.tile_pool(name="feat", bufs=8))
            fr_p = ctx.enter_context(tc.tile_pool(name="fr", bufs=2))
            q_p = ctx.enter_context(tc.tile_pool(name="q", bufs=3))
            s_p = ctx.enter_context(tc.tile_pool(name="s", bufs=2))
            aux_p = ctx.enter_context(tc.tile_pool(name="aux", bufs=3))
            pt_p = ctx.enter_context(tc.tile_pool(name="pt", bufs=2))
            m_p = ctx.enter_context(tc.tile_pool(name="m", bufs=2))
            oi_p = ctx.enter_context(tc.tile_pool(name="oi", bufs=3))
            ps_dist = ctx.enter_context(tc.tile_pool(name="psd", bufs=3, space="PSUM"))
            ps_tr = ctx.enter_context(tc.tile_pool(name="pst", bufs=1, space="PSUM"))
            ps_out = ctx.enter_context(tc.tile_pool(name="pso", bufs=2, space="PSUM"))
            dram_p = ctx.enter_context(tc.tile_pool(name="dram", bufs=2, space="DRAM"))

            # ---- constants (on scalar queue; sync queue is kept for features)
            centT = {}
            for pidx, cdram in ((0, centT0_d), (1, centT1_d)):
                cts = []
                for cc in range(2):
                    ct = const_p.tile([128, 32], f16, tag=f"centT{pidx}_{cc}")
                    nc.scalar.dma_start(
                        out=ct[:], in_=cdram[cc * 128 : (cc + 1) * 128, :]
                    )
                    cts.append(ct)
                centT[pidx] = cts
            csqn = {}
            for pidx, qdram in ((0, csqn0_d), (1, csqn1_d)):
                cq = const_p.tile([128, 1], f32, tag=f"csqn{pidx}")
                nc.scalar.dma_start(out=cq[:], in_=qdram[:, :])
                csqn[pidx] = cq
            wk_sb = const_p.tile([128, K], f32, tag="wk")
            nc.scalar.dma_start(out=wk_sb[:], in_=wk_dram[:, :])
            g_sb = const_p.tile([h, H], bf16, tag="gmat")
            nc.scalar.dma_start(out=g_sb[:], in_=g_dram[:, :])
            ident = const_p.tile([128, 128], f32, tag="ident")
            nc.scalar.dma_start(out=ident[:], in_=ident_dram[:, :])

            class MaskCtx:
                pass

            def new_mask(pidx, feat, out_dram):
                mc = MaskCtx()
                mc.pidx = pidx
                mc.feat = feat
                mc.out_dram = out_dram
                # sg free layout: 32*block + lane (blocks 0:64 from quads);
                # rem region cols 2048:2304 has its own mapping (see rem()).
                mc.sg = s_p.tile([128, NFULL * 512 + REMPAD], f32, tag="sg")
                mc.y = aux_p.tile([128, NFULL * 16], f32, tag="y")
                mc.yr = aux_p.tile([32, 8], f32, tag="yr")
                mc.scratch = dram_p.tile([NT, 128], bf16, tag="scratch")
                mc.msb = m_p.tile([h, w], bf16, tag="m")
                mc.e_sb = m_p.tile([h, W], bf16, tag="e")
                # rows may be read (x0 in G) by gather chunks before the
                # later colexp parts write them — keep them initialized
                nc.gpsimd.memset(mc.e_sb[:], 0.0)
                mc.ft = [None, None]
                return mc

            def load_quad(mc, q):
                for cc in range(2):
                    ft = feat_p.tile([128, QUAD_PX], f16, tag=f"feat{cc}")
                    nc.sync.dma_start(
                        out=ft[:],
                        in_=mc.feat[
                            cc * 128 : (cc + 1) * 128,
                            q * QUAD_PX : (q + 1) * QUAD_PX,
                        ],
                    )
                    mc.ft[cc] = ft

            def mm_quad(mc, q):
                psq = ps_dist.tile([128, 512], f32, tag="dist")
                mc.psq = psq
                for j in range(4):
                    for cc in range(2):
                        moving = mc.ft[cc][:].rearrange(
                            "p (c j2 i) -> p j2 c i", j2=4, i=32
                        )[:, j]
                        nc.tensor.matmul(
                            psq[32 * j : 32 * j + 32, :],
                            centT[mc.pidx][cc][:],
                            moving,
                            start=(cc == 0),
                            stop=(cc == 1),
                            tile_position=(0, 32 * j),
                        )

            def act_st_am(mc, q):
                quad = q_p.tile([128, 512], f32, tag="quad")
                nc.scalar.activation(
                    out=quad[:],
                    in_=mc.psq[:],
                    func=AF.Identity,
                    bias=csqn[mc.pidx][:],
                    scale=2.0,
                )
                sgq = mc.sg[:, 512 * q : 512 * q + 512]
                nc.vector.transpose(out=sgq, in_=quad[:])
                # argmin over the 16 blocks of this quad
                b0, b1 = 16 * q, 16 * q + 16
                sl = mc.sg[:, 32 * b0 : 32 * b1].rearrange(
                    "p (b e) -> p b e", e=32
                )[:, :, 0:K]
                mxt = aux_p.tile([128, 16], f32, tag="mx")
                nc.vector.tensor_reduce(mxt[:], sl, axis=X, op=ALU.max)
                eqt = aux_p.tile([128, 16 * K], f32, tag="eq")
                eqs = eqt[:].rearrange("p (b k) -> p b k", k=K)
                nc.vector.tensor_tensor(
                    out=eqs,
                    in0=sl,
                    in1=mxt[:].unsqueeze(2).broadcast_to([128, 16, K]),
                    op=ALU.is_ge,
                )
                nc.vector.tensor_tensor(
                    out=eqs,
                    in0=eqs,
                    in1=wk_sb[:].unsqueeze(1).broadcast_to([128, 16, K]),
                    op=ALU.mult,
                )
                nc.vector.tensor_reduce(mc.y[:, b0:b1], eqs, axis=X, op=ALU.min)

            def load_rem(mc):
                for cc in range(2):
                    ft = fr_p.tile([128, REMPAD], f16, tag=f"featr{cc}")
                    nc.gpsimd.memset(ft[:, REM:REMPAD], 0.0)
                    nc.sync.dma_start(
                        out=ft[:, 0:REM],
                        in_=mc.feat[cc * 128 : (cc + 1) * 128, NFULL * QUAD_PX : HW],
                    )
                    mc.ft[cc] = ft

            def mm_rem(mc):
                psq = ps_dist.tile([128, 512], f32, tag="dist")
                mc.psq = psq
                for cc in range(2):
                    nc.tensor.matmul(
                        psq[0:32, 0:REMPAD],
                        centT[mc.pidx][cc][:],
                        mc.ft[cc][:],
                        start=(cc == 0),
                        stop=(cc == 1),
                        tile_position=(0, 0),
                    )

            def act_st_am_rem(mc):
                quad = q_p.tile([128, 512], f32, tag="quad")
                nc.scalar.activation(
                    out=quad[0:32, 0:REMPAD],
                    in_=mc.psq[0:32, 0:REMPAD],
                    func=AF.Identity,
                    bias=csqn[mc.pidx][0:32, :],
                    scale=2.0,
                )
                # sg rem region: sg[i, 2048 + 32c + k] = m(px 8192+32c+i, k)
                nc.vector.transpose(
                    out=mc.sg[0:32, NFULL * 512 : NFULL * 512 + REMPAD],
                    in_=quad[0:32, 0:REMPAD],
                )
                sl = mc.sg[0:32, NFULL * 512 : NFULL * 512 + REMPAD].rearrange(
                    "p (b e) -> p b e", e=32
                )[:, :, 0:K]
                mxt = aux_p.tile([128, 16], f32, tag="mx")
                nc.vector.tensor_reduce(mxt[0:32, 0:8], sl, axis=X, op=ALU.max)
                eqt = aux_p.tile([128, 16 * K], f32, tag="eq")
                eqs = eqt[0:32, 0 : 8 * K].rearrange("p (b k) -> p b k", k=K)
                nc.vector.tensor_tensor(
                    out=eqs,
                    in0=sl,
                    in1=mxt[0:32, 0:8].unsqueeze(2).broadcast_to([32, 8, K]),
                    op=ALU.is_ge,
                )
                nc.vector.tensor_tensor(
                    out=eqs,
                    in0=eqs,
                    in1=wk_sb[0:32, :].unsqueeze(1).broadcast_to([32, 8, K]),
                    op=ALU.mult,
                )
                nc.vector.tensor_reduce(mc.yr[:], eqs, axis=X, op=ALU.min)

            def ptt(mc, b0, b1):
                # y blocks [b0,b1) -> scratch flat pixels [128*b0, 128*b1)
                nb = b1 - b0
                ptr = ps_tr.tile([32, 128], f32, tag="tr")
                nc.tensor.transpose(ptr[0:nb, :], mc.y[:, b0:b1], ident[:])
                pttsb = pt_p.tile([32, 128], bf16, tag="pttsb")
                nc.scalar.activation(
                    out=pttsb[0:nb, :], in_=ptr[0:nb, :], func=AF.Copy
                )
                nc.scalar.dma_start(out=mc.scratch[b0:b1, :], in_=pttsb[0:nb, :])

            def ptt_rem(mc):
                # yr [32, 8] -> scratch flat pixels [8192, 8448)
                ptr = ps_tr.tile([32, 128], f32, tag="tr")
                nc.tensor.transpose(ptr[0:8, 0:32], mc.yr[:], ident[0:32, 0:32])
                pttsb = pt_p.tile([32, 128], bf16, tag="pttsb")
                nc.scalar.activation(
                    out=pttsb[0:8, 0:32], in_=ptr[0:8, 0:32], func=AF.Copy
                )
                nc.scalar.dma_start(
                    out=mc.scratch[:]
                    .rearrange("a b -> (a b)")[64 * 128 : 66 * 128]
                    .rearrange("(c i) -> c i", i=32),
                    in_=pttsb[0:8, 0:32],
                )

            def m_dma(mc, r0, r1):
                # issued on the scalar queue right after the scratch writes
                # it depends on: same-queue ordering avoids a cross-queue
                # semaphore round trip
                nc.scalar.dma_start(
                    out=mc.msb[r0:r1, :],
                    in_=mc.scratch[:]
                    .rearrange("a b -> (a b)")[r0 * w : r1 * w]
                    .rearrange("(r c) -> r c", c=w),
                )

            def colexp(mc, r0, r1):
                # column nearest-upsample 129 -> 1024 on rows [r0:r1), fused
                # with +20 so E holds the final class index (bf16).  The
                # nearest-neighbor map floor(c'*129/1024) decomposes into 3
                # affine pieces: head run (srcs 0:16, rep 8), 7 mid runs
                # (15 srcs each, rep 8, src stride 16 / dst stride 127), and
                # 8 singleton srcs 16+16k with rep 7 at dst 128+127k.
                eb = mc.e_sb[r0:r1, :]
                mb = mc.msb[r0:r1, :]

                def eap(off, dims):
                    return AP(eb.tensor, eb.offset + off, [list(eb.ap[0])] + dims)

                def map_(off, dims):
                    return AP(mb.tensor, mb.offset + off, [list(mb.ap[0])] + dims)

                nc.vector.tensor_scalar_add(
                    eap(0, [[8, 16], [1, 8]]), map_(0, [[1, 16], [0, 8]]), 20.0
                )
                nc.vector.tensor_scalar_add(
                    eap(135, [[127, 7], [8, 15], [1, 8]]),
                    map_(17, [[16, 7], [1, 15], [0, 8]]),
                    20.0,
                )
                nc.vector.tensor_scalar_add(
                    eap(128, [[127, 8], [1, 7]]), map_(16, [[16, 8], [0, 7]]), 20.0
                )

            def gather(mc, n, conv_eng="scalar", store_eng="gpsimd"):
                # row nearest-upsample rows [128n, 128n+128) + int8 convert
                oint = oi_p.tile([128, W], i8, tag="oint")
                po = ps_out.tile([128, W], f32, tag="out")
                for hh in range(W // 512):
                    nc.tensor.matmul(
                        po[:, hh * 512 : (hh + 1) * 512],
                        g_sb[:, n * 128 : (n + 1) * 128],
                        mc.e_sb[:, hh * 512 : (hh + 1) * 512],
                        start=True,
                        stop=True,
                    )
                if conv_eng == "scalar":
                    nc.scalar.activation(out=oint[:], in_=po[:], func=AF.Copy)
                else:
                    nc.vector.tensor_copy(out=oint[:], in_=po[:])
                seng = nc.gpsimd if store_eng == "gpsimd" else nc.sync
                seng.dma_start(
                    out=mc.out_dram[n * 128 : (n + 1) * 128, :], in_=oint[:]
                )

            def mask_slot(mc, s):
                # NB: compute-engine partition slices must start at 0/32/64/96;
                # DMA partition ranges are unconstrained.  Row 63 straddles the
                # quad/remainder pixel boundary, so the tail colexp rewrites
                # rows [0,65) once the remainder has landed.
                if s <= 3:
                    load_quad(mc, s)
                    mm_quad(mc, s)
                    if s == 2:
                        ptt(mc, 0, 32)
                    if s == 3:
                        ptt(mc, 32, 48)
                        m_dma(mc, 0, 33)
                    act_st_am(mc, s)
                elif s == 4:
                    # remainder chain first so nothing downstream of the
                    # gathers head-of-line blocks it on the scalar queue
                    load_rem(mc)
                    mm_rem(mc)
                    act_st_am_rem(mc)
                    ptt(mc, 48, 64)
                    ptt_rem(mc)
                    m_dma(mc, 33, h)
                    colexp(mc, 0, 33)
                    gather(mc, 0, conv_eng="scalar")
                    gather(mc, 1, conv_eng="scalar")
                else:
                    colexp(mc, 0, h)
                    gather(mc, 2, conv_eng="vector", store_eng="sync")
                    gather(mc, 3, conv_eng="scalar", store_eng="sync")

            # interleave the two masks' pipelines so both consume the
            # stream window together and only the remainder chains trail
            mc0 = new_mask(0, f_s2t, out0)
            mc1 = new_mask(1, f_tgt, out1)
            for s in range(6):
                mask_slot(mc0, s)
                mask_slot(mc1, s)

    nc.compile()
    return nc


_cached_nc = None


def _get_nc():
    global _cached_nc
    if _cached_nc is None:
        _cached_nc = build_module()
    return _cached_nc


def make_in_maps(feature_s2t, feature_target, centroid_s2t, centroid_target):
    # host-side prep: fp16 features; fp16 transposed padded centroids;
    # per-partition -|c|^2 bias replicated at offsets {0,32,64,96}
    def cent_prep(cent):
        cent = np.asarray(cent, dtype=np.float32)
        centT = np.zeros((C, 32), dtype=np.float16)
        centT[:, 0:K] = cent.T.astype(np.float16)
        csq = (cent * cent).sum(axis=1)
        csqn = np.zeros((128, 1), dtype=np.float32)
        for j in range(4):
            csqn[32 * j : 32 * j + K, 0] = -csq
        return centT, csqn

    # mask0: feature_s2t vs centroid_target; mask1: feature_target vs centroid_s2t
    centT0, csqn0 = cent_prep(centroid_target)
    centT1, csqn1 = cent_prep(centroid_s2t)
    f0 = np.asarray(feature_s2t, dtype=np.float32).reshape(B, C, HW).astype(np.float16)
    f1 = (
        np.asarray(feature_target, dtype=np.float32)
        .reshape(B, C, HW)
        .astype(np.float16)
    )
    in_maps = []
    for b in range(B):
        in_maps.append(
            {
                "feature_s2t": np.ascontiguousarray(f0[b]),
                "feature_target": np.ascontiguousarray(f1[b]),
                "centT0": centT0,
                "centT1": centT1,
                "csqn0": csqn0,
                "csqn1": csqn1,
            }
        )
    return in_maps


def kernel(
    feature_s2t,
    feature_target,
    centroid_s2t,
    centroid_target,
    seg_s2t=None,
    seg_target=None,
    **_unused,
):
    from concourse.bass_utils import run_bass_kernel_spmd

    nc = _get_nc()
    in_maps = make_in_maps(feature_s2t, feature_target, centroid_s2t, centroid_target)
    res = run_bass_kernel_spmd(nc, in_maps, core_ids=list(range(B)))
    results = res.results
    m0 = np.stack([results[b]["out0"] for b in range(B)]).astype(np.int32)
    m1 = np.stack([results[b]["out1"] for b in range(B)]).astype(np.int32)
    return (m0, m1)
